# revision 24
# baseline (speedup 1.0000x reference)
"""Trainium2 Bass kernel for nn_AttentionDecoder (B=64,S=96,H=1024,E=512,V=32000,T=32).

Math restructuring (exact up to fp32 rounding):
  scores[b,s] = Va.(tanh(h Wa^T)) + Va.(keys Ua^T)  -- softmax over s drops the
  first term (constant per row), so attention weights w = softmax_s(keys.u) with
  u = Ua^T Va are CONSTANT across decode steps; ctx is hoisted entirely.
  GRU: gi = emb W_ihE^T + [ctx W_ihC^T + b_ih] (second part hoisted as gctx).

Distribution (8 cores):
  - attention prologue: batch-sharded (8 rows/core) + AllGather of ctx
  - GRU gates: hidden-sharded (128 of 1024 per core), h_new AllGathered per step
  - output projection: vocab-sharded (4000 rows/core, SBUF-resident);
    per-step cross-core argmax via AllGather of per-core top-1 candidates
  - greedy loop fully unrolled (32 steps), fp32 matmuls everywhere so the
    argmax trajectory matches the fp32 reference.
"""
import os
import numpy as np

import concourse.bass as bass
import concourse.bacc as bacc
import concourse.mybir as mybir
import concourse.tile as tile
from concourse.bass_utils import run_bass_kernel_spmd

F32 = mybir.dt.float32
U32 = mybir.dt.uint32
AF = mybir.ActivationFunctionType
OP = mybir.AluOpType
AX = mybir.AxisListType

NCORES = 8
B, S, H, E, V, T = 64, 96, 1024, 512, 32000, 32
START = 1
BS = B // NCORES          # 8 batch rows per core (attention prologue)
VS = V // NCORES          # 4000 vocab rows per core
HS = H // NCORES          # 128 hidden units per core
KT = H // 128             # 8 contraction tiles of 128
VHALF = VS // 2           # 2000
NCHUNK = 4                # logits free chunks per half (4 x 500)
CH = VHALF // NCHUNK      # 500
BIG = 1.0e9


def build_nc(trace_scopes=False, debug=False):
    nc = bacc.Bacc(None, target_bir_lowering=False, num_devices=NCORES)
    dbg_outs = {}
    if debug:
        for nm, shp in [("d_emb", [B, E]), ("d_embT", [128, 4 * B]),
                        ("d_trz", [128, 2 * B]), ("d_ghnb", [128, B]),
                        ("d_ginc", [128, B]), ("d_rz", [128, 2 * B]),
                        ("d_n", [128, B]), ("d_hsl", [HS, B]),
                        ("d_gctx", [128, 3 * B]), ("d_hT0", [128, KT * B])]:
            dbg_outs[nm] = nc.dram_tensor(nm, shp, F32, kind="ExternalOutput")

    # ---------------- kernel I/O ----------------
    keys_in = nc.dram_tensor("keys_in", [BS, S, H], F32, kind="ExternalInput")
    h0T_in = nc.dram_tensor("h0T_in", [H, B], F32, kind="ExternalInput")
    h0sel_in = nc.dram_tensor("h0sel_in", [HS, B], F32, kind="ExternalInput")
    embtab_in = nc.dram_tensor("embtab_in", [V, E], F32, kind="ExternalInput")
    Ua_in = nc.dram_tensor("Ua_in", [H, H], F32, kind="ExternalInput")
    Va_in = nc.dram_tensor("Va_in", [H, 1], F32, kind="ExternalInput")
    WhhT_in = nc.dram_tensor("WhhT_in", [H, 3 * HS], F32, kind="ExternalInput")
    WihET_in = nc.dram_tensor("WihET_in", [E, 3 * HS], F32, kind="ExternalInput")
    WihCT_in = nc.dram_tensor("WihCT_in", [H, 3 * HS], F32, kind="ExternalInput")
    WoutT_in = nc.dram_tensor("WoutT_in", [H, VS], F32, kind="ExternalInput")
    bias2k_in = nc.dram_tensor("bias2k_in", [128, VHALF], F32, kind="ExternalInput")
    bih_in = nc.dram_tensor("bih_in", [HS, 3], F32, kind="ExternalInput")
    bhh_in = nc.dram_tensor("bhh_in", [HS, 3], F32, kind="ExternalInput")
    vbase_in = nc.dram_tensor("vbase_in", [128, 1], F32, kind="ExternalInput")
    ident_in = nc.dram_tensor("ident_in", [128, 128], F32, kind="ExternalInput")

    out_logits = nc.dram_tensor("out_logits", [B, T, VS], F32, kind="ExternalOutput")
    out_attn = nc.dram_tensor("out_attn", [BS, T, S], F32, kind="ExternalOutput")
    out_hT = nc.dram_tensor("out_hT", [128, KT, B], F32, kind="ExternalOutput")

    rg = [list(range(NCORES))]

    from contextlib import ExitStack
    with tile.TileContext(nc, num_cores=NCORES) as tc, ExitStack() as stack:
        if True:
            wp = stack.enter_context(tc.tile_pool(name="wpool", bufs=1))
            lp = stack.enter_context(tc.tile_pool(name="lpool", bufs=1))
            dp = stack.enter_context(tc.tile_pool(name="dpool", bufs=1, space="DRAM"))
            # ------------- persistent weight loads -------------
            ident_sb = wp.tile([128, 128], F32)
            nc.sync.dma_start(ident_sb[:], ident_in[:])
            WhhT_sb = wp.tile([128, KT, 3 * HS], F32)
            nc.sync.dma_start(WhhT_sb[:], WhhT_in.rearrange("(t p) m -> p t m", p=128))
            WihET_sb = wp.tile([128, 4, 3 * HS], F32)
            nc.sync.dma_start(WihET_sb[:], WihET_in.rearrange("(t p) m -> p t m", p=128))
            bias2k_sb = wp.tile([128, VHALF], F32)
            nc.sync.dma_start(bias2k_sb[:], bias2k_in[:])
            bih_sb = wp.tile([HS, 3], F32)
            nc.sync.dma_start(bih_sb[:], bih_in[:])
            bhh_sb = wp.tile([HS, 3], F32)
            nc.sync.dma_start(bhh_sb[:], bhh_in[:])
            vbase_sb = wp.tile([128, 1], F32)
            nc.sync.dma_start(vbase_sb[:], vbase_in[:])
            gctx_sb = wp.tile([128, 3, B], F32)

            # collective bounce buffers
            ag1_in = dp.tile([HS, B], F32)
            ag2_in = dp.tile([128, 2], F32)
            agc_in = dp.tile([128, KT * BS], F32)
            agc_out = dp.tile([NCORES, 128, KT * BS], F32, addr_space="Shared")
            u_dram = dp.tile([KT, 128], F32)

            # ---------------- prologue: attention hoist ----------------
            with (
                tc.tile_pool(name="ppool", bufs=1) as pp,
                tc.tile_pool(name="ppsum", bufs=1, space="PSUM") as pps,
            ):
                keys_sb = pp.tile([S, BS, H], F32)
                nc.sync.dma_start(keys_sb[:], keys_in.rearrange("b s h -> s b h"))
                Ua_sb = pp.tile([128, KT, H], F32)
                nc.sync.dma_start(Ua_sb[:], Ua_in.rearrange("(t p) m -> p t m", p=128))
                Va_sb = pp.tile([128, KT, 1], F32)
                nc.sync.dma_start(Va_sb[:], Va_in.rearrange("(t p) o -> p t o", p=128))
                WihCT_sb = pp.tile([128, KT, 3 * HS], F32)
                nc.sync.dma_start(WihCT_sb[:], WihCT_in.rearrange("(t p) m -> p t m", p=128))

                # u = Ua^T Va  -> [H] as [128, KT]
                psum_u = pps.tile([128, KT], F32)
                for mt in range(KT):
                    for kt in range(KT):
                        nc.tensor.matmul(
                            psum_u[:, mt:mt + 1],
                            Ua_sb[:, kt, mt * 128:(mt + 1) * 128],
                            Va_sb[:, kt, :],
                            start=(kt == 0), stop=(kt == KT - 1),
                        )
                u_col = pp.tile([128, KT], F32)
                nc.vector.tensor_copy(u_col[:], psum_u[:])
                # store transposed -> u_dram[m, p] so flat order is h = m*128+p
                nc.sync.dma_start(u_dram.rearrange("m p -> p m"), u_col[:])
                u_row = pp.tile([1, H], F32)
                nc.sync.dma_start(u_row[:], u_dram.rearrange("(o m) p -> o (m p)", o=1))
                u_bc = pp.tile([S, H], F32)
                nc.gpsimd.partition_broadcast(u_bc[:], u_row[:])

                # scores^T [S, BS]
                scT = pp.tile([S, BS], F32)
                tmp_su = pp.tile([S, H], F32)
                for b in range(BS):
                    nc.vector.tensor_tensor(tmp_su[:], keys_sb[:, b, :], u_bc[:], op=OP.mult)
                    nc.vector.tensor_reduce(scT[:, b:b + 1], tmp_su[:], axis=AX.X, op=OP.add)

                # transpose -> [BS, S], softmax along free axis
                psum_t = pps.tile([BS, S], F32)
                nc.tensor.transpose(psum_t[:], scT[:], ident_sb[0:S, 0:S])
                sc_bs = pp.tile([BS, S], F32)
                nc.vector.tensor_copy(sc_bs[:], psum_t[:])
                mx8 = pp.tile([BS, 1], F32)
                nc.vector.tensor_reduce(mx8[:], sc_bs[:], axis=AX.X, op=OP.max)
                nmx8 = pp.tile([BS, 1], F32)
                nc.vector.tensor_scalar_mul(nmx8[:], mx8[:], -1.0)
                e_bs = pp.tile([BS, S], F32)
                nc.scalar.activation(e_bs[:], sc_bs[:], AF.Exp, bias=nmx8[:, 0:1])
                den = pp.tile([BS, 1], F32)
                nc.vector.tensor_reduce(den[:], e_bs[:], axis=AX.X, op=OP.add)
                rden = pp.tile([BS, 1], F32)
                nc.vector.reciprocal(rden[:], den[:])
                w_bs = pp.tile([BS, S], F32)
                nc.vector.tensor_scalar(w_bs[:], e_bs[:], rden[:, 0:1], None, op0=OP.mult)

                # attentions output: broadcast over T
                attn_bc = pp.tile([BS, T, S], F32)
                nc.vector.tensor_copy(attn_bc[:], w_bs.rearrange("(o p) s -> p o s", o=1).to_broadcast([BS, T, S]))
                nc.sync.dma_start(out_attn[:], attn_bc[:])

                # w^T [S, BS] for ctx matmuls
                psum_w = pps.tile([S, BS], F32)
                nc.tensor.transpose(psum_w[:], w_bs[:], ident_sb[0:BS, 0:BS])
                wT = pp.tile([S, BS], F32)
                nc.vector.tensor_copy(wT[:], psum_w[:])

                # ctx^T slice: [128, KT, BS]
                psum_ctx = pps.tile([128, KT, BS], F32)
                for b in range(BS):
                    for mt in range(KT):
                        nc.tensor.matmul(
                            psum_ctx[:, mt, b:b + 1],
                            keys_sb[:, b, mt * 128:(mt + 1) * 128],
                            wT[:, b:b + 1],
                            start=True, stop=True,
                        )
                ctx_sl = pp.tile([128, KT * BS], F32)
                nc.vector.tensor_copy(ctx_sl[:], psum_ctx.rearrange("p t b -> p (t b)"))
                nc.sync.dma_start(agc_in[:], ctx_sl[:])
                nc.gpsimd.collective_compute(
                    "AllGather", OP.bypass, replica_groups=rg,
                    ins=[agc_in[:]], outs=[agc_out[:]],
                )
                # reassemble full ctx^T [128, KT, B];  global b = r*BS + b_local
                ctxT = pp.tile([128, KT, NCORES, BS], F32)
                nc.sync.dma_start(
                    ctxT[:],
                    agc_out.rearrange("r p (t b) -> p t r b", t=KT),
                )

                # gctx = W_ihC_sel^T tiles @ ctx^T  (+ biases)
                psum_gc = pps.tile([128, 3, B], F32)
                for mt in range(3):
                    for kt in range(KT):
                        nc.tensor.matmul(
                            psum_gc[:, mt, :],
                            WihCT_sb[:, kt, mt * 128:(mt + 1) * 128],
                            ctxT[:, kt, :, :],
                            start=(kt == 0), stop=(kt == KT - 1),
                        )
                # r,z gates: fold b_ih + b_hh;  n gate: fold b_ih only
                for mt in range(2):
                    nc.vector.tensor_scalar(
                        gctx_sb[:, mt, :], psum_gc[:, mt, :],
                        bih_sb[:, mt:mt + 1], bhh_sb[:, mt:mt + 1],
                        op0=OP.add, op1=OP.add,
                    )
                nc.vector.tensor_scalar(
                    gctx_sb[:, 2, :], psum_gc[:, 2, :],
                    bih_sb[:, 2:3], None, op0=OP.add,
                )

            # big weight load after prologue pool closes (SBUF peak control)
            wop = stack.enter_context(tc.tile_pool(name="wop", bufs=1))
            lps = stack.enter_context(tc.tile_pool(name="lpsum", bufs=1, space="PSUM"))
            WoutT_sb = wop.tile([128, KT, VS], F32)
            nc.sync.dma_start(WoutT_sb[:], WoutT_in.rearrange("(t p) n -> p t n", p=128))

            # ---------------- decode loop ----------------
            psum_rz = lps.tile([128, 2, B], F32)
            psum_ghn = lps.tile([128, B], F32)
            psum_gin = lps.tile([128, B], F32)
            psum_tr = lps.tile([128, 4, B], F32)
            psum_log = lps.tile([128, NCHUNK, 512], F32)

            hT_prev = lp.tile([128, KT, B], F32, name="hT", tag="hT", bufs=2)
            nc.sync.dma_start(hT_prev[:], h0T_in.rearrange("(t p) b -> p t b", p=128))
            hsl_prev = lp.tile([HS, B], F32, name="hsl", tag="hsl", bufs=2)
            nc.sync.dma_start(hsl_prev[:], h0sel_in[:])
            tok_u = lp.tile([B, 1], U32, name="tok_u", tag="tok", bufs=2)
            nc.vector.memset(tok_u[:], START)

            def emit_logits_argmax(t, hT_cur, do_argmax):
                """logits(t) = W_out_shard @ h_full(t) in [128, 2000] layout
                (partitions 0:64 -> vocab 0:2000, 64:128 -> vocab 2000:4000)."""
                log_sb = lp.tile([128, VHALF], F32, name=f"log_{t}", tag="log", bufs=2)
                for c in range(NCHUNK):
                    for kt in range(KT):
                        nc.tensor.matmul(
                            psum_log[0:B, c, 0:CH],
                            hT_cur[:, kt, :],
                            WoutT_sb[:, kt, c * CH:(c + 1) * CH],
                            start=(kt == 0), stop=(kt == KT - 1),
                            tile_position=(0, 0),
                        )
                    for kt in range(KT):
                        nc.tensor.matmul(
                            psum_log[B:128, c, 0:CH],
                            hT_cur[:, kt, :],
                            WoutT_sb[:, kt, VHALF + c * CH:VHALF + (c + 1) * CH],
                            start=(kt == 0), stop=(kt == KT - 1),
                            tile_position=(0, B),
                        )
                    nc.vector.tensor_tensor(
                        log_sb[:, c * CH:(c + 1) * CH],
                        psum_log[:, c, 0:CH],
                        bias2k_sb[:, c * CH:(c + 1) * CH],
                        op=OP.add,
                    )
                # stream out this step's logits
                nc.sync.dma_start(out_logits[:, t, 0:VHALF], log_sb[0:B, :])
                nc.sync.dma_start(out_logits[:, t, VHALF:VS], log_sb[B:128, :])
                if not do_argmax:
                    return None

                mxv = lp.tile([128, 8], F32, name=f"mxv_{t}", tag="mxv", bufs=2)
                mxi = lp.tile([128, 8], U32, name=f"mxi_{t}", tag="mxi", bufs=2)
                nc.vector.max_with_indices(mxv[:], mxi[:], log_sb[:])
                pay = lp.tile([128, 2], F32, name=f"pay_{t}", tag="pay", bufs=2)
                nc.vector.tensor_copy(pay[:, 0:1], mxv[:, 0:1])
                mif = lp.tile([128, 1], F32, name=f"mif_{t}", tag="mif", bufs=2)
                nc.vector.tensor_copy(mif[:], mxi[:, 0:1])  # cast u32 -> f32
                nc.vector.tensor_scalar(pay[:, 1:2], mif[:], vbase_sb[:, 0:1], None, op0=OP.add)
                nc.sync.dma_start(ag2_in[:], pay[:])
                ag2_out = dp.tile([NCORES, 128, 2], F32, addr_space="Shared",
                                  name=f"ag2o_{t}")
                nc.gpsimd.collective_compute(
                    "AllGather", OP.bypass, replica_groups=rg,
                    ins=[ag2_in[:]], outs=[ag2_out[:]],
                )
                # cand[b, (r, half, v)] <- ag2_out[r, half*64 + b, v]
                cand = lp.tile([B, NCORES, 2, 2], F32, name=f"cand_{t}", tag="cand", bufs=2)
                nc.sync.dma_start(
                    cand[:],
                    ag2_out.rearrange("r (h b) v -> b r h v", h=2),
                )
                top = lp.tile([B, 1], F32, name=f"top_{t}", tag="top", bufs=2)
                nc.vector.tensor_reduce(top[:], cand[:, :, :, 0], axis=AX.XY, op=OP.max)
                msk = lp.tile([B, NCORES, 2], F32, name=f"msk_{t}", tag="msk", bufs=2)
                nc.vector.tensor_scalar(msk[:], cand[:, :, :, 0], top[:, 0:1], None, op0=OP.is_equal)
                mg = lp.tile([B, NCORES, 2], F32, name=f"mg_{t}", tag="mg", bufs=2)
                nc.vector.tensor_tensor(mg[:], msk[:], cand[:, :, :, 1], op=OP.mult)
                inv = lp.tile([B, NCORES, 2], F32, name=f"inv_{t}", tag="inv", bufs=2)
                nc.vector.tensor_scalar(inv[:], msk[:], -BIG, BIG, op0=OP.mult, op1=OP.add)
                mgi = lp.tile([B, NCORES, 2], F32, name=f"mgi_{t}", tag="mgi", bufs=2)
                nc.vector.tensor_tensor(mgi[:], mg[:], inv[:], op=OP.add)
                tok_f = lp.tile([B, 1], F32, name=f"tokf_{t}", tag="tokf", bufs=2)
                nc.vector.tensor_reduce(tok_f[:], mgi[:], axis=AX.XY, op=OP.min)
                tk = lp.tile([B, 1], U32, name=f"toku_{t}", tag="tok", bufs=2)
                nc.vector.tensor_copy(tk[:], tok_f[:])  # cast f32 -> u32
                return tk

            for t in range(T):
                # gh_n(t): W_hh_sel^T n-tiles @ h_full_in(t)   [uses previous AG1]
                for kt in range(KT):
                    nc.tensor.matmul(
                        psum_ghn[:],
                        WhhT_sb[:, kt, 2 * 128:3 * 128],
                        hT_prev[:, kt, :],
                        start=(kt == 0), stop=(kt == KT - 1),
                    )

                # logits/argmax of previous step + token for this step
                if t > 0:
                    tok_u = emit_logits_argmax(t - 1, hT_cur, do_argmax=True)

                # embedding gather + relu
                emb_sb = lp.tile([B, E], F32, name=f"emb_{t}", tag="emb", bufs=2)
                nc.gpsimd.indirect_dma_start(
                    out=emb_sb[:],
                    out_offset=None,
                    in_=embtab_in[:],
                    in_offset=bass.IndirectOffsetOnAxis(ap=tok_u[:, 0:1], axis=0),
                )
                embr = lp.tile([B, E], F32, name=f"embr_{t}", tag="embr", bufs=2)
                nc.vector.tensor_scalar_max(embr[:], emb_sb[:], 0.0)
                # transpose to [128, 4, B]
                for k in range(4):
                    nc.tensor.transpose(
                        psum_tr[:, k, :], embr[:, k * 128:(k + 1) * 128],
                        ident_sb[0:B, 0:B],
                    )
                embT = lp.tile([128, 4, B], F32, name=f"embT_{t}", tag="embT", bufs=2)
                nc.vector.tensor_copy(embT[:], psum_tr[:])

                # r,z gates: ONE contiguous 12-tile accumulation group each
                # (8x W_hh @ hT  +  4x W_ihE @ embT)
                for mt in range(2):
                    for j in range(KT + 4):
                        if j < KT:
                            lhsT = WhhT_sb[:, j, mt * 128:(mt + 1) * 128]
                            rhs = hT_prev[:, j, :]
                        else:
                            lhsT = WihET_sb[:, j - KT, mt * 128:(mt + 1) * 128]
                            rhs = embT[:, j - KT, :]
                        nc.tensor.matmul(
                            psum_rz[:, mt, :], lhsT, rhs,
                            start=(j == 0), stop=(j == KT + 3),
                        )
                # gi_n
                for kt in range(4):
                    nc.tensor.matmul(
                        psum_gin[:],
                        WihET_sb[:, kt, 2 * 128:3 * 128],
                        embT[:, kt, :],
                        start=(kt == 0), stop=(kt == 3),
                    )

                # gates
                tmp_rz = lp.tile([128, 2, B], F32, name=f"trz_{t}", tag="trz", bufs=2)
                nc.vector.tensor_tensor(tmp_rz[:], psum_rz[:], gctx_sb[:, 0:2, :], op=OP.add)
                rz = lp.tile([128, 2, B], F32, name=f"rz_{t}", tag="rz", bufs=2)
                nc.scalar.activation(rz[:], tmp_rz[:], AF.Sigmoid)
                ghn_b = lp.tile([128, B], F32, name=f"ghnb_{t}", tag="ghnb", bufs=2)
                nc.vector.tensor_scalar(ghn_b[:], psum_ghn[:], bhh_sb[:, 2:3], None, op0=OP.add)
                rghn = lp.tile([128, B], F32, name=f"rghn_{t}", tag="rghn", bufs=2)
                nc.vector.tensor_tensor(rghn[:], rz[:, 0, :], ghn_b[:], op=OP.mult)
                ginc = lp.tile([128, B], F32, name=f"ginc_{t}", tag="ginc", bufs=2)
                nc.vector.tensor_tensor(ginc[:], psum_gin[:], gctx_sb[:, 2, :], op=OP.add)
                npre = lp.tile([128, B], F32, name=f"npre_{t}", tag="npre", bufs=2)
                nc.vector.tensor_tensor(npre[:], rghn[:], ginc[:], op=OP.add)
                n_sb = lp.tile([128, B], F32, name=f"n_{t}", tag="n", bufs=2)
                nc.scalar.activation(n_sb[:], npre[:], AF.Tanh)
                dmn = lp.tile([128, B], F32, name=f"dmn_{t}", tag="dmn", bufs=2)
                nc.vector.tensor_tensor(dmn[:], hsl_prev[:], n_sb[:], op=OP.subtract)
                zd = lp.tile([128, B], F32, name=f"zd_{t}", tag="zd", bufs=2)
                nc.vector.tensor_tensor(zd[:], rz[:, 1, :], dmn[:], op=OP.mult)
                hsl = lp.tile([HS, B], F32, name=f"hsl_{t}", tag="hsl", bufs=2)
                nc.vector.tensor_tensor(hsl[:], n_sb[:], zd[:], op=OP.add)

                if debug and t == 0:
                    for nm, src in [("d_emb", emb_sb), ("d_embT", embT),
                                    ("d_trz", tmp_rz), ("d_ghnb", ghn_b),
                                    ("d_ginc", ginc), ("d_rz", rz),
                                    ("d_n", n_sb), ("d_hsl", hsl),
                                    ("d_gctx", gctx_sb), ("d_hT0", hT_prev)]:
                        s = src[:]
                        if len(s.shape) == 3:
                            s = s.rearrange("p a b -> p (a b)")
                        nc.sync.dma_start(dbg_outs[nm][:], s)

                # AG1: gather h_new slices -> full h^T
                nc.sync.dma_start(ag1_in[:], hsl[:])
                ag1_out = dp.tile([NCORES, HS, B], F32, addr_space="Shared",
                                  name=f"ag1o_{t}")
                nc.gpsimd.collective_compute(
                    "AllGather", OP.bypass, replica_groups=rg,
                    ins=[ag1_in[:]], outs=[ag1_out[:]],
                )
                hT_cur = lp.tile([128, KT, B], F32, name=f"hT_{t}", tag="hT", bufs=2)
                nc.sync.dma_start(
                    hT_cur[:], ag1_out.rearrange("r p b -> p r b"),
                )
                hT_prev = hT_cur
                hsl_prev = hsl

            # final step logits (no argmax) + h_T output
            emit_logits_argmax(T - 1, hT_cur, do_argmax=False)
            nc.sync.dma_start(out_hT[:], hT_cur[:])

    nc.compile()
    return nc


_CACHE = {}


def _get_nc():
    if "nc" not in _CACHE:
        _CACHE["nc"] = build_nc()
    return _CACHE["nc"]


def kernel(encoder_outputs, encoder_hidden, emb_table, Wa, Ua, Va,
           W_ih, W_hh, b_ih, b_hh, W_out, b_out, max_length):
    assert int(max_length) == T
    f = np.ascontiguousarray
    enc = np.asarray(encoder_outputs, np.float32)
    h0 = np.asarray(encoder_hidden, np.float32)[0]            # [B, H]
    h0T = f(h0.T)                                             # [H, B]
    emb_tab = f(np.asarray(emb_table, np.float32))
    Ua = np.asarray(Ua, np.float32)
    Va = np.asarray(Va, np.float32).reshape(H, 1)
    W_ih = np.asarray(W_ih, np.float32)
    W_hh = np.asarray(W_hh, np.float32)
    b_ih = np.asarray(b_ih, np.float32)
    b_hh = np.asarray(b_hh, np.float32)
    W_out = np.asarray(W_out, np.float32)
    b_out = np.asarray(b_out, np.float32)
    ident = np.eye(128, dtype=np.float32)

    in_maps = []
    for c in range(NCORES):
        sel = np.r_[HS * c:HS * (c + 1),
                    H + HS * c:H + HS * (c + 1),
                    2 * H + HS * c:2 * H + HS * (c + 1)]
        vlo, vhi = VS * c, VS * (c + 1)
        bias2k = np.concatenate([
            np.broadcast_to(b_out[vlo:vlo + VHALF], (64, VHALF)),
            np.broadcast_to(b_out[vlo + VHALF:vhi], (64, VHALF)),
        ], axis=0)
        vbase = (VS * c + (np.arange(128) // 64) * VHALF).astype(np.float32)
        in_maps.append({
            "keys_in": f(enc[BS * c:BS * (c + 1)]),
            "h0T_in": h0T,
            "h0sel_in": f(h0T[HS * c:HS * (c + 1)]),
            "embtab_in": emb_tab,
            "Ua_in": Ua,
            "Va_in": f(Va),
            "WhhT_in": f(W_hh[sel].T),
            "WihET_in": f(W_ih[sel, :E].T),
            "WihCT_in": f(W_ih[sel, E:].T),
            "WoutT_in": f(W_out[vlo:vhi].T),
            "bias2k_in": f(bias2k),
            "bih_in": f(b_ih[sel].reshape(3, HS).T),
            "bhh_in": f(b_hh[sel].reshape(3, HS).T),
            "vbase_in": f(vbase.reshape(128, 1)),
            "ident_in": ident,
        })

    nc = _get_nc()
    res = run_bass_kernel_spmd(
        nc, in_maps, core_ids=list(range(NCORES)),
        trace=bool(int(os.environ.get("KBENCH_TRACE", "0"))),
    )
    _CACHE["last_results"] = res

    decoder_outputs = np.concatenate(
        [res.results[c]["out_logits"] for c in range(NCORES)], axis=2)
    attentions = np.concatenate(
        [res.results[c]["out_attn"] for c in range(NCORES)], axis=0)
    hT = res.results[0]["out_hT"]                              # [128, KT, B]
    h_T = hT.transpose(2, 1, 0).reshape(B, H)[None]            # [1, B, H]
    return decoder_outputs, h_T, attentions
